# revision 1
# baseline (speedup 1.0000x reference)
"""Trainium2 Bass kernel for fused causal-shift cross-entropy loss.

Problem: hidden_states [4, 2048, 2048] f32, lm_head_weight [32000, 2048] f32,
labels [4, 2048] int. Reference: causal shift, logits = h @ W^T, mean NLL.

Strategy (token data-parallel, no collectives):
  - N = 4*2047 = 8188 shifted tokens, padded to 8192 = 8 cores x 1024 tokens.
  - Each core computes sumexp_n = sum_v exp(h_n . W_v) for its 1024 tokens
    over the full vocab (32000), via bf16 matmul (f32 PSUM accumulation) and
    a fused exp+row-sum on the scalar engine. Logits are ~N(0,1) here (max
    |logit| ~ 7), so exp without max-subtraction is safe in f32; the kernel
    output is checked finite on host.
  - Host computes logit_at_label exactly (f64 row dot), then
    loss = mean(log(sumexp_n) - logit_label_n) over valid tokens.

The heavy compute (1.07 TFLOP matmul) runs on the PE arrays of all 8 cores;
exp/reduce hide under the matmul. Host-side work is O(N*D) = 0.003% of flops.
"""

import os
import sys
import types

import numpy as np
import ml_dtypes


# ---- shim: image's antenv lacks axon_hooks; provide it so NTFF tracing works
def _install_ntff_hook():
    try:
        import antenv

        try:
            from antenv.axon_hooks import get_axon_ntff_profile_hook  # noqa: F401

            return
        except ImportError:
            pass
        from trn_agent_boot.trn_boot import _ntff_profile_via_ctypes

        hook = _ntff_profile_via_ctypes("/opt/axon/libaxon_pjrt.so")
        mod = types.ModuleType("antenv.axon_hooks")
        mod._hook = hook
        mod.get_axon_ntff_profile_hook = lambda: mod._hook
        mod.set_axon_ntff_profile_hook = lambda h: setattr(mod, "_hook", h)
        sys.modules["antenv.axon_hooks"] = mod
        antenv.axon_hooks = mod
    except Exception as e:  # pragma: no cover - profiling is best-effort
        print("ntff hook shim failed:", e, file=sys.stderr)


_install_ntff_hook()

import concourse.bass as bass  # noqa: E402
import concourse.mybir as mybir  # noqa: E402
import concourse.tile as tile  # noqa: E402
from concourse import bacc  # noqa: E402
from concourse.bass_utils import run_bass_kernel_spmd  # noqa: E402

NCORES = 8
P = 128          # SBUF/PSUM partitions
D = 2048         # hidden dim
KT = D // P      # 16 k-chunks of 128
T = 1024         # tokens per core (8192 padded / 8 cores)
TT = T // P      # 8 token tiles per core
V = 32000        # vocab
VT = 500         # vocab tile (columns per matmul; PSUM bank holds 512 f32)
NV = V // VT     # 64 vocab tiles

IGNORE_INDEX = -100

_COMPILED = None          # cached (nc,) across kernel() calls in one process
LAST_RESULTS = None       # BassKernelResults of the most recent run (for test.py)


def _build():
    nc = bacc.Bacc("TRN2", target_bir_lowering=False, debug=False,
                   num_devices=NCORES)
    bf16 = mybir.dt.bfloat16
    f32 = mybir.dt.float32

    ht = nc.dram_tensor("ht", [D, T], bf16, kind="ExternalInput").ap()
    wt = nc.dram_tensor("wt", [D, V], bf16, kind="ExternalInput").ap()
    out = nc.dram_tensor("out", [P, TT], f32, kind="ExternalOutput").ap()

    with tile.TileContext(nc) as tc:
        with (
            tc.tile_pool(name="hpool", bufs=1) as hpool,
            tc.tile_pool(name="wpool", bufs=3) as wpool,
            tc.tile_pool(name="ppool", bufs=6, space="PSUM") as ppool,
            tc.tile_pool(name="epool", bufs=4) as epool,
            tc.tile_pool(name="apool", bufs=1) as apool,
        ):
            # resident activations: [p, k, t] with d = k*128 + p
            ht_s = hpool.tile([P, KT, T], bf16)
            nc.sync.dma_start(out=ht_s[:], in_=ht.rearrange("(k p) t -> p k t", p=P))

            # per-(token-tile, vocab-tile) partial row sums of exp(logits)
            acc = apool.tile([P, TT, NV], f32)

            for vi in range(NV):
                w_s = wpool.tile([P, KT, VT], bf16)
                nc.sync.dma_start(
                    out=w_s[:],
                    in_=wt[:, vi * VT:(vi + 1) * VT].rearrange(
                        "(k p) v -> p k v", p=P),
                )
                for ti in range(TT):
                    ps = ppool.tile([P, VT], f32)
                    for k in range(KT):
                        nc.tensor.matmul(
                            ps[:],
                            ht_s[:, k, ti * P:(ti + 1) * P],
                            w_s[:, k, :],
                            start=(k == 0),
                            stop=(k == KT - 1),
                        )
                    ex = epool.tile([P, VT], f32)
                    nc.scalar.activation(
                        ex[:], ps[:], mybir.ActivationFunctionType.Exp,
                        accum_out=acc[:, ti, vi:vi + 1],
                    )

            red = apool.tile([P, TT], f32)
            nc.vector.tensor_reduce(
                red[:], acc[:], axis=mybir.AxisListType.X, op=mybir.AluOpType.add,
            )
            nc.sync.dma_start(out=out[:], in_=red[:])

    nc.compile()
    return nc


def kernel(hidden_states, lm_head_weight, labels):
    global _COMPILED, LAST_RESULTS

    h3 = np.asarray(hidden_states, dtype=np.float32)
    w = np.asarray(lm_head_weight, dtype=np.float32)
    lab = np.asarray(labels)

    B, S, Dh = h3.shape
    assert (Dh, w.shape) == (D, (V, D)), (h3.shape, w.shape)

    h = h3[:, :-1, :].reshape(-1, Dh)          # [N, D]
    t = lab[:, 1:].reshape(-1)                 # [N]
    N = h.shape[0]
    NPAD = NCORES * T
    assert N <= NPAD

    if _COMPILED is None:
        _COMPILED = _build()
    nc = _COMPILED

    # device inputs: h^T per core (bf16), W^T shared (bf16)
    hp = np.zeros((NPAD, Dh), np.float32)
    hp[:N] = h
    ht_all = np.ascontiguousarray(hp.T.astype(ml_dtypes.bfloat16))  # [D, NPAD]
    wt = np.ascontiguousarray(w.T.astype(ml_dtypes.bfloat16))       # [D, V]
    in_maps = [
        {"ht": np.ascontiguousarray(ht_all[:, c * T:(c + 1) * T]), "wt": wt}
        for c in range(NCORES)
    ]

    trace = os.environ.get("KERNEL_TRACE", "0") == "1"
    res = run_bass_kernel_spmd(
        nc, in_maps, core_ids=list(range(NCORES)), trace=trace,
    )
    LAST_RESULTS = res

    # out[p, ti] holds token ti*128 + p of that core
    sumexp = np.concatenate(
        [res.results[c]["out"].T.reshape(-1) for c in range(NCORES)]
    )[:N].astype(np.float64)
    assert np.isfinite(sumexp).all() and (sumexp > 0).all()

    # exact logit at label on host (tiny: N*D flops)
    valid = t != IGNORE_INDEX
    safe_t = np.where(valid, t, 0).astype(np.int64)
    wrows = w[safe_t].astype(np.float64)                   # [N, D]
    ll = np.einsum("nd,nd->n", h.astype(np.float64), wrows)

    nll = np.log(sumexp) - ll
    nll = np.where(valid, nll, 0.0)
    n_valid = max(int(valid.sum()), 1)
    return np.float32(nll.sum() / n_valid)


# revision 5
# speedup vs baseline: 2.3213x; 2.3213x over previous
"""Trainium2 Bass kernel for fused causal-shift cross-entropy loss.

Problem: hidden_states [4, 2048, 2048] f32, lm_head_weight [32000, 2048] f32,
labels [4, 2048] int. Reference: causal shift, logits = h @ W^T, mean NLL.

Strategy (token data-parallel, no collectives):
  - N = 4*2047 = 8188 shifted tokens, padded to 8192 = 8 cores x 1024 tokens.
  - Each core computes sumexp_n = sum_v exp(h_n . W_v) for its 1024 tokens
    over the full vocab (32000), via bf16 matmul (f32 PSUM accumulation) and
    a fused exp+row-sum on the scalar engine. Logits are ~N(0,1) here (max
    |logit| ~ 7), so exp without max-subtraction is safe in f32; the kernel
    output is checked finite on host.
  - Host computes logit_at_label exactly (f64 row dot), then
    loss = mean(log(sumexp_n) - logit_label_n) over valid tokens.

The heavy compute (1.07 TFLOP matmul) runs on the PE arrays of all 8 cores;
exp/reduce hide under the matmul. Host-side work is O(N*D) = 0.003% of flops.
"""

import os
import sys
import types

import numpy as np
import ml_dtypes


# ---- shim: image's antenv lacks axon_hooks; provide it so NTFF tracing works
def _install_ntff_hook():
    try:
        import antenv

        try:
            from antenv.axon_hooks import get_axon_ntff_profile_hook  # noqa: F401

            return
        except ImportError:
            pass
        from trn_agent_boot.trn_boot import _ntff_profile_via_ctypes

        hook = _ntff_profile_via_ctypes("/opt/axon/libaxon_pjrt.so")
        mod = types.ModuleType("antenv.axon_hooks")
        mod._hook = hook
        mod.get_axon_ntff_profile_hook = lambda: mod._hook
        mod.set_axon_ntff_profile_hook = lambda h: setattr(mod, "_hook", h)
        sys.modules["antenv.axon_hooks"] = mod
        antenv.axon_hooks = mod
    except Exception as e:  # pragma: no cover - profiling is best-effort
        print("ntff hook shim failed:", e, file=sys.stderr)


_install_ntff_hook()

import concourse.bass as bass  # noqa: E402
import concourse.mybir as mybir  # noqa: E402
import concourse.tile as tile  # noqa: E402
from concourse import bacc  # noqa: E402
from concourse.bass_utils import run_bass_kernel_spmd  # noqa: E402

NCORES = 8
P = 128          # SBUF/PSUM partitions
D = 2048         # hidden dim
KT = D // P      # 16 k-chunks of 128
T = 1024         # tokens per core (8192 padded / 8 cores)
TT = T // P      # 8 token tiles per core
V = 32000        # vocab
VT = 500         # vocab tile (columns per matmul; PSUM bank holds 512 f32)
NV = V // VT     # 64 vocab tiles

# fp8 e4m3 matmul at DoubleRow (2x) rate. W is pre-scaled by W_SCALE on host
# so its values (std ~0.022) leave e4m3's denormal range; the matmul then
# produces W_SCALE * logits and the scalar engine computes
# exp(psum / W_SCALE) via its free input scale.
USE_FP8 = True
W_SCALE = 64.0

IGNORE_INDEX = -100

_COMPILED = None          # cached (nc,) across kernel() calls in one process
LAST_RESULTS = None       # BassKernelResults of the most recent run (for test.py)


def _build():
    nc = bacc.Bacc("TRN2", target_bir_lowering=False, debug=False,
                   num_devices=NCORES)
    mmdt = mybir.dt.float8e4 if USE_FP8 else mybir.dt.bfloat16
    f32 = mybir.dt.float32

    ht = nc.dram_tensor("ht", [D, T], mmdt, kind="ExternalInput").ap()
    wt = nc.dram_tensor("wt", [D, V], mmdt, kind="ExternalInput").ap()
    out = nc.dram_tensor("out", [P, TT], f32, kind="ExternalOutput").ap()

    with tile.TileContext(nc) as tc:
        with (
            tc.tile_pool(name="hpool", bufs=1) as hpool,
            tc.tile_pool(name="wpool", bufs=3) as wpool,
            tc.tile_pool(name="ppool", bufs=6, space="PSUM") as ppool,
            tc.tile_pool(name="epool", bufs=4) as epool,
            tc.tile_pool(name="apool", bufs=1) as apool,
        ):
            # resident activations: [p, k, t] with d = k*128 + p
            ht_s = hpool.tile([P, KT, T], mmdt)
            nc.sync.dma_start(out=ht_s[:], in_=ht.rearrange("(k p) t -> p k t", p=P))

            # per-(token-tile, vocab-tile) partial row sums of exp(logits)
            acc = apool.tile([P, TT, NV], f32)

            kstep = 2 if USE_FP8 else 1
            perf_mode = mybir.MatmulPerfMode.DoubleRow if USE_FP8 else None
            exp_scale = (1.0 / W_SCALE) if USE_FP8 else 1.0

            for vi in range(NV):
                w_s = wpool.tile([P, KT, VT], mmdt)
                nc.sync.dma_start(
                    out=w_s[:],
                    in_=wt[:, vi * VT:(vi + 1) * VT].rearrange(
                        "(k p) v -> p k v", p=P),
                )
                for ti in range(TT):
                    ps = ppool.tile([P, VT], f32)
                    for k in range(0, KT, kstep):
                        if USE_FP8:
                            lhsT = ht_s[:, k:k + 2, ti * P:(ti + 1) * P]
                            rhs = w_s[:, k:k + 2, :]
                        else:
                            lhsT = ht_s[:, k, ti * P:(ti + 1) * P]
                            rhs = w_s[:, k, :]
                        nc.tensor.matmul(
                            ps[:], lhsT, rhs,
                            start=(k == 0),
                            stop=(k + kstep >= KT),
                            perf_mode=perf_mode,
                        )
                    ex = epool.tile([P, VT], f32)
                    nc.scalar.activation(
                        ex[:], ps[:], mybir.ActivationFunctionType.Exp,
                        scale=exp_scale,
                        accum_out=acc[:, ti, vi:vi + 1],
                    )

            red = apool.tile([P, TT], f32)
            nc.vector.tensor_reduce(
                red[:], acc[:], axis=mybir.AxisListType.X, op=mybir.AluOpType.add,
            )
            nc.sync.dma_start(out=out[:], in_=red[:])

    nc.compile()
    return nc


def kernel(hidden_states, lm_head_weight, labels):
    global _COMPILED, LAST_RESULTS

    h3 = np.asarray(hidden_states, dtype=np.float32)
    w = np.asarray(lm_head_weight, dtype=np.float32)
    lab = np.asarray(labels)

    B, S, Dh = h3.shape
    assert (Dh, w.shape) == (D, (V, D)), (h3.shape, w.shape)

    h = h3[:, :-1, :].reshape(-1, Dh)          # [N, D]
    t = lab[:, 1:].reshape(-1)                 # [N]
    N = h.shape[0]
    NPAD = NCORES * T
    assert N <= NPAD

    if _COMPILED is None:
        _COMPILED = _build()
    nc = _COMPILED

    # device inputs: h^T per core, W^T shared
    hp = np.zeros((NPAD, Dh), np.float32)
    hp[:N] = h
    if USE_FP8:
        mmdt_np = ml_dtypes.float8_e4m3
        ht_all = np.ascontiguousarray(
            np.clip(hp.T, -240.0, 240.0).astype(mmdt_np))            # [D, NPAD]
        wt = np.ascontiguousarray(
            np.clip(w.T * W_SCALE, -240.0, 240.0).astype(mmdt_np))   # [D, V]
    else:
        ht_all = np.ascontiguousarray(hp.T.astype(ml_dtypes.bfloat16))
        wt = np.ascontiguousarray(w.T.astype(ml_dtypes.bfloat16))
    in_maps = [
        {"ht": np.ascontiguousarray(ht_all[:, c * T:(c + 1) * T]), "wt": wt}
        for c in range(NCORES)
    ]

    trace = os.environ.get("KERNEL_TRACE", "0") == "1"
    res = run_bass_kernel_spmd(
        nc, in_maps, core_ids=list(range(NCORES)), trace=trace,
    )
    LAST_RESULTS = res

    # out[p, ti] holds token ti*128 + p of that core
    sumexp = np.concatenate(
        [res.results[c]["out"].T.reshape(-1) for c in range(NCORES)]
    )[:N].astype(np.float64)
    assert np.isfinite(sumexp).all() and (sumexp > 0).all()

    # exact logit at label on host (tiny: N*D flops)
    valid = t != IGNORE_INDEX
    safe_t = np.where(valid, t, 0).astype(np.int64)
    wrows = w[safe_t].astype(np.float64)                   # [N, D]
    ll = np.einsum("nd,nd->n", h.astype(np.float64), wrows)

    nll = np.log(sumexp) - ll
    nll = np.where(valid, nll, 0.0)
    n_valid = max(int(valid.sum()), 1)
    return np.float32(nll.sum() / n_valid)
